# revision 1
# baseline (speedup 1.0000x reference)
"""Bass/Tile kernel for nn_CustomCrossAttnProcessor (8-core data-parallel).

Each NeuronCore processes one batch element (B=8 == n_cores).
Per-core compute, one batch element:
  q = hs @ w_q                     (f32r matmuls, N=256)
  k/v = enc @ w_{k,v}; ip_k/ip_v = ip @ w_{k,v}_ip
  scoresT[t, s] per head, exp (no max-subtract: |scores*scale| <= ~5)
  PV with ones-column appended to v -> softmax denominators for free
  norm_ipa via per-token stats, out = hs_sum @ w_out + b_out
"""
import sys

for _p in ("/opt/trn_rl_repo",):
    if _p not in sys.path:
        sys.path.append(_p)

from contextlib import ExitStack

import numpy as np

import concourse.bass as bass  # noqa: F401
import concourse.tile as tile
import concourse.mybir as mybir
from concourse import bass_utils, bacc
from concourse.bass import ts, ds
from concourse.masks import make_identity

B, S, D = 8, 4096, 1280
T, P_IP, C = 77, 16, 2048
H, HD = 20, 64
N_CORES = 8
SB = 256            # tokens per s-block
NBLK = S // SB      # 16
SCALE = HD ** -0.5  # 0.125
EPS = 1e-7
KD = D // 128       # 10
KC = C // 128       # 16
CAT = 112           # rows: txt probs [0:77], gap [77:96], ip probs [96:112]
IPOFF, TXTOFF = 96, 0
VW = HD + 2         # 66: v cols + ones col (softmax sum) + v-rowsum col (mean)
ALU = mybir.AluOpType
FT = mybir.ActivationFunctionType

f32 = mybir.dt.float32
f32r = mybir.dt.float32r
bf16 = mybir.dt.bfloat16

_CACHE = {}


def _build():
    nc = bacc.Bacc(
        "TRN2", target_bir_lowering=False, debug=False, enable_asserts=False,
        num_devices=N_CORES,
    )
    hs_d = nc.dram_tensor("hidden_states", [S, D], f32, kind="ExternalInput").ap()
    enc_d = nc.dram_tensor("encoder_hidden_states", [T, C], f32,
                           kind="ExternalInput").ap()
    ip_d = nc.dram_tensor("ip_hidden_states", [P_IP, C], f32,
                          kind="ExternalInput").ap()
    wq_d = nc.dram_tensor("w_q", [D, D], f32r, kind="ExternalInput").ap()
    wk_d = nc.dram_tensor("w_k", [C, D], f32r, kind="ExternalInput").ap()
    wv_d = nc.dram_tensor("w_v", [C, D], f32r, kind="ExternalInput").ap()
    wkip_d = nc.dram_tensor("w_k_ip", [C, D], f32r, kind="ExternalInput").ap()
    wvip_d = nc.dram_tensor("w_v_ip", [C, D], f32r, kind="ExternalInput").ap()
    wout_d = nc.dram_tensor("w_out", [D, D], f32r, kind="ExternalInput").ap()
    bout_d = nc.dram_tensor("b_out", [D], f32, kind="ExternalInput").ap()
    out_d = nc.dram_tensor("out", [S, D], f32, kind="ExternalOutput").ap()

    with tile.TileContext(nc) as tc, ExitStack() as ctx:
        n = tc.nc
        const = ctx.enter_context(tc.tile_pool(name="const", bufs=1))
        wq_sb = const.tile([128, KD, D], f32r)
        wout_sb = const.tile([128, KD, D], f32r)
        bias_sb = const.tile([128, D], f32)
        ktc_sb = const.tile([128, KD, 93], f32r)  # txt cols 0:77, ip 77:93
        vaug_sb = const.tile([128, H, VW], bf16)     # rows 32:109 hold v + ones
        ipv_sb = const.tile([P_IP, H, VW], bf16)
        ident = const.tile([128, 128], f32)
        ones_col = const.tile([1, 128], f32)
        b_row = const.tile([1, D], f32)

        make_identity(n, ident[:])
        n.vector.memset(ones_col[:], 1.0)
        n.vector.memset(vaug_sb[TXTOFF:TXTOFF + T, :, HD:HD + 1], 1.0)
        n.vector.memset(ipv_sb[:, :, HD:HD + 1], 1.0)
        n.sync.dma_start(wq_sb[:], wq_d.rearrange("(ko ki) m -> ki ko m", ki=128))
        n.sync.dma_start(wout_sb[:], wout_d.rearrange("(ko ki) m -> ki ko m", ki=128))
        n.sync.dma_start(b_row[:], bout_d[None, :])

        # ---------------- setup: bias replicate, k/v projections ----------
        with tc.tile_pool(name="setup", bufs=1) as setup, \
             tc.tile_pool(name="setup_w", bufs=3) as setup_w:
          with tc.tile_pool(name="sps1", bufs=2, space="PSUM") as sps1:
            for j in range(3):
                w = min(512, D - j * 512)
                bp = sps1.tile([128, 512], f32, tag="bp")
                n.tensor.matmul(bp[:, :w], ones_col[:], b_row[:, ds(j * 512, w)],
                                start=True, stop=True)
                n.vector.tensor_copy(bias_sb[:, ds(j * 512, w)], bp[:, :w])

            enc_sb = setup.tile([T, C], f32, tag="enc")
            n.sync.dma_start(enc_sb[:], enc_d)
            encT = setup.tile([128, KC, T], f32r, tag="encT")
            for c in range(KC):
                tp = sps1.tile([128, T], f32, tag="tp")
                n.tensor.transpose(tp[:], enc_sb[:, ts(c, 128)], ident[:T, :T])
                n.vector.tensor_copy(encT[:, c, :], tp[:])
            ipx_sb = setup.tile([P_IP, C], f32, tag="ipx")
            n.sync.dma_start(ipx_sb[:], ip_d)
            ipT = setup.tile([128, KC, P_IP], f32r, tag="ipT")
            for c in range(KC):
                tp = sps1.tile([128, T], f32, tag="tp")
                n.tensor.transpose(tp[:, :P_IP], ipx_sb[:, ts(c, 128)],
                                   ident[:P_IP, :P_IP])
                n.vector.tensor_copy(ipT[:, c, :], tp[:, :P_IP])

            # kT_cat: txt at cols 32:109, ip at cols 0:16
            for dt_ in range(KD):
                wk_t = setup_w.tile([128, KC, 128], f32r, tag="wk")
                n.sync.dma_start(
                    wk_t[:],
                    wk_d.rearrange("(co ci) m -> ci co m", ci=128)[:, :, ts(dt_, 128)])
                kp = sps1.tile([128, T], f32, tag="kp")
                for c in range(KC):
                    n.tensor.matmul(kp[:], wk_t[:, c, :].bitcast(f32),
                                    encT[:, c, :].bitcast(f32),
                                    start=(c == 0), stop=(c == KC - 1))
                n.vector.tensor_copy(ktc_sb[:, dt_, 0:T], kp[:])
                wkip_t = setup_w.tile([128, KC, 128], f32r, tag="wk")
                n.sync.dma_start(
                    wkip_t[:],
                    wkip_d.rearrange("(co ci) m -> ci co m", ci=128)[:, :, ts(dt_, 128)])
                kp2 = sps1.tile([128, T], f32, tag="kp")
                for c in range(KC):
                    n.tensor.matmul(kp2[:, :P_IP], wkip_t[:, c, :].bitcast(f32),
                                    ipT[:, c, :].bitcast(f32),
                                    start=(c == 0), stop=(c == KC - 1))
                n.vector.tensor_copy(ktc_sb[:, dt_, T:T + P_IP], kp2[:, :P_IP])

          with tc.tile_pool(name="sps2", bufs=1, space="PSUM") as sps2:
            if True:
                vp = sps2.tile([T, 3 * 512], f32, tag="vp")
                ivp = sps2.tile([P_IP, 3 * 512], f32, tag="ivp")
                for c in range(KC):
                    wv_c = setup_w.tile([128, D], f32r, tag="wv")
                    n.sync.dma_start(
                        wv_c[:],
                        wv_d.rearrange("(co ci) m -> ci co m", ci=128)[:, c, :])
                    wvip_c = setup_w.tile([128, D], f32r, tag="wv")
                    n.sync.dma_start(
                        wvip_c[:],
                        wvip_d.rearrange("(co ci) m -> ci co m", ci=128)[:, c, :])
                    for j in range(3):
                        w = min(512, D - j * 512)
                        n.tensor.matmul(vp[:, ds(j * 512, w)], encT[:, c, :],
                                        wv_c[:, ds(j * 512, w)],
                                        start=(c == 0), stop=(c == KC - 1))
                        n.tensor.matmul(
                            ivp[:, ds(j * 512, w)], ipT[:, c, :],
                            wvip_c[:, ds(j * 512, w)],
                            start=(c == 0), stop=(c == KC - 1))
                # scatter [77, 1280] -> vaug [77, 20, 0:64]
                n.vector.tensor_copy(
                    vaug_sb[TXTOFF:TXTOFF + T, :, 0:HD],
                    vp[:, :D].rearrange("p (h c) -> p h c", c=HD))
                n.vector.tensor_copy(
                    ipv_sb[:, :, 0:HD],
                    ivp[:, :D].rearrange("p (h c) -> p h c", c=HD))
                with n.allow_low_precision(reason="v row-sums feed small mean "
                                           "correction; bf16 is plenty"):
                    n.vector.reduce_sum(
                        vaug_sb[TXTOFF:TXTOFF + T, :, HD + 1:HD + 2],
                        vp[:, :D].rearrange("p (h c) -> p h c", c=HD),
                        axis=mybir.AxisListType.X)
                    n.vector.reduce_sum(
                        ipv_sb[:, :, HD + 1:HD + 2],
                        ivp[:, :D].rearrange("p (h c) -> p h c", c=HD),
                        axis=mybir.AxisListType.X)

        # ---------------- main loop over s-blocks --------------------------
        lp = ctx.enter_context(tc.tile_pool(name="lp", bufs=2))
        lp1 = ctx.enter_context(tc.tile_pool(name="lp1", bufs=1))
        lps = ctx.enter_context(tc.tile_pool(name="lps", bufs=1))
        lpo = ctx.enter_context(tc.tile_pool(name="lpo", bufs=2))
        lpp = ctx.enter_context(tc.tile_pool(name="lpp", bufs=6))
        ps_mm = ctx.enter_context(tc.tile_pool(name="ps_mm", bufs=2, space="PSUM"))
        ps_tr = ctx.enter_context(tc.tile_pool(name="ps_tr", bufs=2, space="PSUM"))
        ps_sc = ctx.enter_context(tc.tile_pool(name="ps_sc", bufs=2, space="PSUM"))
        ps_pv = ctx.enter_context(tc.tile_pool(name="ps_pv", bufs=2, space="PSUM"))

        BANKS = [list(range(6 * g, min(H, 6 * g + 6))) for g in range(4)]

        for b in range(NBLK):
            s0 = b * SB
            # load + transpose hs -> hsT [d, s]
            hsT = lp1.tile([128, KD, SB], f32r, tag="hsT")
            hs_t = {}
            for si in range(2):
                hs_t[si] = lp.tile([128, D], f32, tag="hs", name=f"hs{si}")
                n.sync.dma_start(hs_t[si][:], hs_d[ds(s0 + si * 128, 128), :])
            for dp in range(0, KD, 2):
                tp = ps_tr.tile([128, 512], f32, tag="tr")
                for dd in range(2):
                    for si in range(2):
                        n.tensor.transpose(tp[:, ds(dd * 256 + si * 128, 128)],
                                           hs_t[si][:, ts(dp + dd, 128)], ident[:])
                n.vector.tensor_copy(hsT[:, dp:dp + 2, :], tp[:])
            # qT [d, s]
            qT = lp1.tile([128, KD, SB], f32r, tag="qT")
            for dp in range(0, KD, 2):
                qp = ps_mm.tile([128, 512], f32, tag="mm")
                for dd in range(2):
                    for k in range(KD):
                        n.tensor.matmul(qp[:, ds(dd * SB, SB)],
                                        wq_sb[:, k, ts(dp + dd, 128)], hsT[:, k, :],
                                        start=(k == 0), stop=(k == KD - 1))
                n.vector.tensor_copy(qT[:, dp:dp + 2, :], qp[:])

            lat = lp1.tile([128, 2, D], f32, tag="lat")
            ipo = lp1.tile([128, 2, D], f32, tag="ipo")
            msum_l = lps.tile([128, 2, H], f32, tag="msl")
            msum_i = lps.tile([128, 2, H], f32, tag="msi")
            sm_l = lps.tile([128, 2, H, 2], f32, tag="ssl")
            sm_i = lps.tile([128, 2, H, 2], f32, tag="ssi")
            recip_l = lps.tile([128, 2, H], f32, tag="rcl")
            recip_i = lps.tile([128, 2, H], f32, tag="rci")
            st = lps.tile([128, 2, 16], f32, tag="st")

            for g, bank in enumerate(BANKS):
                pT = {}
                pTi = {}
                for h in bank:
                    dt_, half = h // 2, h % 2
                    sc = ps_sc.tile([T, 2 * SB], f32, tag="sc")
                    n.tensor.matmul(sc[:, 0:SB],
                                    ktc_sb[ds(64 * half, 64), dt_, 0:T],
                                    qT[ds(64 * half, 64), dt_, :],
                                    start=True, stop=True)
                    n.tensor.matmul(sc[0:P_IP, ds(SB, SB)],
                                    ktc_sb[ds(64 * half, 64), dt_, T:T + P_IP],
                                    qT[ds(64 * half, 64), dt_, :],
                                    start=True, stop=True)
                    pT[h] = lpp.tile([T, SB], bf16, tag="pT", name=f"pT{h}")
                    pTi[h] = lpp.tile([P_IP, SB], bf16, tag="pTi", name=f"pTi{h}")
                    n.scalar.activation(pT[h][:], sc[:, 0:SB], FT.Exp, scale=SCALE)
                    n.scalar.activation(pTi[h][:], sc[0:P_IP, ds(SB, SB)],
                                        FT.Exp, scale=SCALE)
                nb = len(bank)
                for si in range(2):
                    for br in range(2):  # 0 = txt, 1 = ip
                        pv = ps_pv.tile([128, 6 * VW], f32, tag="pv")
                        for bi, h in enumerate(bank):
                            if br == 0:
                                lhsT = pT[h][:, ts(si, 128)]
                                rhs = vaug_sb[0:T, h, :]
                            else:
                                lhsT = pTi[h][:, ts(si, 128)]
                                rhs = ipv_sb[:, h, :]
                            n.tensor.matmul(pv[:, ds(bi * VW, VW)], lhsT, rhs,
                                            start=True, stop=True)
                        sm = sm_l if br == 0 else sm_i
                        recip = recip_l if br == 0 else recip_i
                        msum = msum_l if br == 0 else msum_i
                        dest = lat if br == 0 else ipo
                        pv3 = pv[:, :nb * VW].rearrange("p (h c) -> p h c", c=VW)
                        n.vector.tensor_copy(
                            sm[:, si, ds(6 * g, nb), :], pv3[:, :, HD:HD + 2])
                        n.vector.reciprocal(recip[:, si, ds(6 * g, nb)],
                                            sm[:, si, ds(6 * g, nb), 0])
                        n.vector.tensor_mul(msum[:, si, ds(6 * g, nb)],
                                            sm[:, si, ds(6 * g, nb), 1],
                                            recip[:, si, ds(6 * g, nb)])
                        n.vector.tensor_tensor(
                            dest[:, si, ds(6 * g * HD, nb * HD)].rearrange(
                                "p (h c) -> p h c", c=HD),
                            pv3[:, :, 0:HD],
                            recip[:, si, ds(6 * g, nb), None].to_broadcast(
                                [128, nb, HD]),
                            op=ALU.mult)

            # ---- norm_ipa stats + combine + out projection ----
            hsT2 = lp1.tile([128, KD, SB], f32r, tag="hsT2")
            for si in range(2):
                scr = lps.tile([128, D], f32, tag="scr")
                n.vector.reduce_sum(st[:, si, 0:1], msum_l[:, si, :],
                                    axis=mybir.AxisListType.X)
                n.vector.reduce_sum(st[:, si, 1:2], msum_i[:, si, :],
                                    axis=mybir.AxisListType.X)
                n.vector.scalar_tensor_tensor(
                    out=scr[:], in0=lat[:, si, :], scalar=1.0, in1=lat[:, si, :],
                    op0=ALU.mult, op1=ALU.mult, accum_out=st[:, si, 2:3])
                n.vector.scalar_tensor_tensor(
                    out=scr[:], in0=ipo[:, si, :], scalar=1.0, in1=ipo[:, si, :],
                    op0=ALU.mult, op1=ALU.mult, accum_out=st[:, si, 3:4])
                n.vector.tensor_scalar_mul(st[:, si, 4:5], st[:, si, 0:1], 1.0 / D)
                n.vector.tensor_scalar_mul(st[:, si, 5:6], st[:, si, 1:2], 1.0 / D)
                n.vector.tensor_mul(st[:, si, 6:7], st[:, si, 4:5], st[:, si, 4:5])
                n.vector.tensor_mul(st[:, si, 7:8], st[:, si, 5:6], st[:, si, 5:6])
                n.vector.tensor_scalar(out=st[:, si, 8:9], in0=st[:, si, 2:3],
                                       scalar1=1.0 / D, scalar2=st[:, si, 6:7],
                                       op0=ALU.mult, op1=ALU.subtract)
                n.vector.tensor_scalar(out=st[:, si, 9:10], in0=st[:, si, 3:4],
                                       scalar1=1.0 / D, scalar2=st[:, si, 7:8],
                                       op0=ALU.mult, op1=ALU.subtract)
            # std = sqrt(var) via DVE: fast-inverse-sqrt init + 3 Newton iters
            # (keeps ACT on the Exp table all kernel long)
            i32 = mybir.dt.int32
            vv = st[:, :, 8:10]
            yy = st[:, :, 10:12]
            t0 = st[:, :, 12:14]
            n.vector.tensor_scalar(out=yy.bitcast(i32), in0=vv.bitcast(i32),
                                   scalar1=1, scalar2=None,
                                   op0=ALU.logical_shift_right)
            n.vector.tensor_scalar(out=yy.bitcast(i32), in0=yy.bitcast(i32),
                                   scalar1=-1, scalar2=0x5f3759df,
                                   op0=ALU.mult, op1=ALU.add)
            for _ in range(3):
                n.vector.tensor_mul(t0[:], yy[:], yy[:])
                n.vector.tensor_mul(t0[:], t0[:], vv[:])
                n.vector.tensor_scalar(out=t0[:], in0=t0[:], scalar1=-0.5,
                                       scalar2=1.5, op0=ALU.mult, op1=ALU.add)
                n.vector.tensor_mul(yy[:], yy[:], t0[:])
            # y ~= rsqrt(var); std = var * y
            n.vector.tensor_mul(yy[:], vv[:], yy[:])
            for si in range(2):
                n.vector.tensor_scalar_add(st[:, si, 12:13], st[:, si, 11:12], EPS)
                n.vector.reciprocal(st[:, si, 13:14], st[:, si, 12:13])
                n.vector.tensor_mul(st[:, si, 14:15], st[:, si, 10:11], st[:, si, 13:14])
                # gneg = alpha*mean_ip - mean_lat
                n.vector.scalar_tensor_tensor(
                    out=st[:, si, 15:16], in0=st[:, si, 5:6], scalar=st[:, si, 14:15],
                    in1=st[:, si, 4:5], op0=ALU.mult, op1=ALU.subtract)
                # hs_sum = lat + alpha*ip - gneg
                n.vector.scalar_tensor_tensor(
                    out=lat[:, si, :], in0=ipo[:, si, :],
                    scalar=st[:, si, 14:15], in1=lat[:, si, :],
                    op0=ALU.mult, op1=ALU.add)
                n.vector.tensor_scalar_sub(lat[:, si, :], lat[:, si, :],
                                           st[:, si, 15:16])
            for dp in range(0, KD, 2):
                tp = ps_tr.tile([128, 512], f32, tag="tr")
                for dd in range(2):
                    for si in range(2):
                        n.tensor.transpose(tp[:, ds(dd * 256 + si * 128, 128)],
                                           lat[:, si, ts(dp + dd, 128)], ident[:])
                n.vector.tensor_copy(hsT2[:, dp:dp + 2, :], tp[:])
            for si in range(2):
                for j in range(3):
                    w = min(512, D - j * 512)
                    op = ps_mm.tile([128, 512], f32, tag="mm")
                    for k in range(KD):
                        n.tensor.matmul(op[:, :w], hsT2[:, k, ts(si, 128)],
                                        wout_sb[:, k, ds(j * 512, w)],
                                        start=(k == 0), stop=(k == KD - 1))
                    ost = lpo.tile([128, 512], f32, tag="ost")
                    n.vector.tensor_add(ost[:, :w], op[:, :w],
                                        bias_sb[:, ds(j * 512, w)])
                    n.sync.dma_start(
                        out_d[ds(s0 + si * 128, 128), ds(j * 512, w)], ost[:, :w])
    nc.compile()
    return nc


def _get_nc():
    if "nc" not in _CACHE:
        _CACHE["nc"] = _build()
    return _CACHE["nc"]


def kernel(**inputs) -> np.ndarray:
    nc = _get_nc()
    f = lambda x: np.ascontiguousarray(np.asarray(x), dtype=np.float32)
    shared = {k: f(inputs[k]) for k in
              ("w_q", "w_k", "w_v", "w_k_ip", "w_v_ip", "w_out", "b_out")}
    hs = f(inputs["hidden_states"])
    enc = f(inputs["encoder_hidden_states"])
    ipx = f(inputs["ip_hidden_states"])
    in_maps = [
        dict(shared, hidden_states=hs[i], encoder_hidden_states=enc[i],
             ip_hidden_states=ipx[i])
        for i in range(N_CORES)
    ]
    res = bass_utils.run_bass_kernel_spmd(nc, in_maps, core_ids=list(range(N_CORES)))
    return np.stack([res.results[i]["out"] for i in range(N_CORES)], axis=0)


if __name__ == "__main__":
    rng = np.random.default_rng(0)
    ins = {
        "hidden_states": rng.standard_normal((B, S, D), dtype=np.float32),
        "encoder_hidden_states": rng.standard_normal((B, T, C), dtype=np.float32),
        "ip_hidden_states": rng.standard_normal((B, P_IP, C), dtype=np.float32),
        "w_q": (rng.standard_normal((D, D), dtype=np.float32) * 0.02),
        "w_k": (rng.standard_normal((C, D), dtype=np.float32) * 0.02),
        "w_v": (rng.standard_normal((C, D), dtype=np.float32) * 0.02),
        "w_k_ip": (rng.standard_normal((C, D), dtype=np.float32) * 0.02),
        "w_v_ip": (rng.standard_normal((C, D), dtype=np.float32) * 0.02),
        "w_out": (rng.standard_normal((D, D), dtype=np.float32) * 0.02),
        "b_out": np.zeros((D,), dtype=np.float32),
    }
    out = kernel(**ins)
    print("out", out.shape, out.dtype, float(np.abs(out).max()))



# revision 14
# speedup vs baseline: 1.4367x; 1.4367x over previous
"""Bass/Tile kernel for nn_CustomCrossAttnProcessor (8-core data-parallel).

Each NeuronCore processes one batch element (B=8 == n_cores).
Software-pipelined across 256-token s-blocks:
  iter i emits: out-projection for block i-1 (interleaved into the score
  loop so the PE never stalls on the stats chain), attention for block i
  (scores+exp packed txt|ip in a 112-row layout, PV with ones/rowsum
  columns), then hs-load/transpose + q-proj for block i+FILLD.
Engine split: PE matmuls; ACT exp/psum-copies/squares; DVE psum-touching
normalize/combine + stats chain; Pool (gpsimd) SBUF-only elementwise +
weight converts.
"""
import sys

for _p in ("/opt/trn_rl_repo",):
    if _p not in sys.path:
        sys.path.append(_p)

from contextlib import ExitStack

import numpy as np

import concourse.bass as bass  # noqa: F401
import concourse.tile as tile
import concourse.mybir as mybir
from concourse import bass_utils, bacc
from concourse.bass import ts, ds
from concourse.masks import make_identity

B, S, D = 8, 4096, 1280
T, P_IP, C = 77, 16, 2048
H, HD = 20, 64
SB = 256            # tokens per s-block
NBLK = S // SB      # 16
SCALE = HD ** -0.5  # 0.125
EPS = 1e-7
KD = D // 128       # 10
KC = C // 128       # 16
CAT = 112           # rows: txt [0:77], gap [77:96], ip [96:112]
IPOFF = 96
VW = HD + 2         # 66: v cols + ones col (softmax sum) + v-rowsum col
FILLD = 5           # q-proj pipeline depth (blocks hoisted over weight DMA)
ALU = mybir.AluOpType
FT = mybir.ActivationFunctionType

f32 = mybir.dt.float32
f32r = mybir.dt.float32r
bf16 = mybir.dt.bfloat16

_CACHE = {}


def _build():
    nc = bacc.Bacc(
        "TRN2", target_bir_lowering=False, debug=False, enable_asserts=False,
        num_devices=8,
    )
    hs_d = nc.dram_tensor("hidden_states", [S, D], f32, kind="ExternalInput").ap()
    enc_d = nc.dram_tensor("encoder_hidden_states", [T, C], f32,
                           kind="ExternalInput").ap()
    ip_d = nc.dram_tensor("ip_hidden_states", [P_IP, C], f32,
                          kind="ExternalInput").ap()
    wq_d = nc.dram_tensor("w_q", [D, D], f32r, kind="ExternalInput").ap()
    wk_d = nc.dram_tensor("w_k", [C, D], f32r, kind="ExternalInput").ap()
    wv_d = nc.dram_tensor("w_v", [C, D], f32r, kind="ExternalInput").ap()
    wkip_d = nc.dram_tensor("w_k_ip", [C, D], f32r, kind="ExternalInput").ap()
    wvip_d = nc.dram_tensor("w_v_ip", [C, D], f32r, kind="ExternalInput").ap()
    wout_d = nc.dram_tensor("w_out", [D, D], f32, kind="ExternalInput").ap()
    bout_d = nc.dram_tensor("b_out", [D], f32, kind="ExternalInput").ap()
    out_d = nc.dram_tensor("out", [S, D], f32, kind="ExternalOutput").ap()

    with tile.TileContext(nc) as tc, ExitStack() as ctx:
        n = tc.nc
        const = ctx.enter_context(tc.tile_pool(name="const", bufs=1))
        wq_sb = const.tile([128, KD, D], bf16)
        wout_bf = const.tile([128, KD, D], bf16)
        ktc_sb = const.tile([128, KD, CAT], bf16)
        vcat = const.tile([CAT, H, VW], bf16)
        ident = const.tile([128, 128], f32)
        ident_bf = const.tile([128, 128], bf16)
        ones_bf = const.tile([1, 128], bf16)
        bias_bf = const.tile([1, D], bf16)

        make_identity(n, ident[:])
        make_identity(n, ident_bf[:])
        n.gpsimd.memset(ones_bf[:], 1.0)
        n.gpsimd.memset(ktc_sb[:, :, T:IPOFF], 0.0)
        n.vector.memset(vcat[0:T, :, HD:HD + 1], 1.0)
        n.vector.memset(vcat[IPOFF:CAT, :, HD:HD + 1], 1.0)


        # ---------------- loop pools (lazy alloc; live through the loop) ---
        lp = ctx.enter_context(tc.tile_pool(name="lp", bufs=2))       # hs
        lpq = ctx.enter_context(tc.tile_pool(name="lpq", bufs=FILLD))  # qT
        lph = ctx.enter_context(tc.tile_pool(name="lph", bufs=1))     # hsT
        lscr = ctx.enter_context(tc.tile_pool(name="lscr", bufs=1))   # sq scratch
        lp1 = ctx.enter_context(tc.tile_pool(name="lp1", bufs=2))     # lat/ipo/hsT2
        lps = ctx.enter_context(tc.tile_pool(name="lps", bufs=2))     # stats
        lpo = ctx.enter_context(tc.tile_pool(name="lpo", bufs=2))     # ost
        lpp = ctx.enter_context(tc.tile_pool(name="lpp", bufs=20))    # pT
        # single shared ring of 1KB psum slots (bank-granular allocator):
        # hs/enc/k transposes, q-proj accumulators, score tiles, tr2 tiles
        ps_u = ctx.enter_context(tc.tile_pool(name="ps_u", bufs=3, space="PSUM"))

        hs_tiles = {}

        def emit_load(b):
            if b >= NBLK:
                return
            for si in range(2):
                t_ = lp.tile([128, D], f32, tag="hs", name=f"hs{b}_{si}")
                n.sync.dma_start(t_[:], hs_d[ds(b * SB + si * 128, 128), :])
                hs_tiles[(b, si)] = t_

        qT_tiles = {}

        def emit_fill(b):
            """hs transposes + q projection for block b -> qT (bf16)."""
            if b >= NBLK:
                return
            hsT = lph.tile([128, KD, SB], bf16, tag="hsT")
            for dp in range(KD):
                tp = ps_u.tile([128, SB], f32, tag="u")
                for si in range(2):
                    n.tensor.transpose(tp[:, ts(si, 128)],
                                       hs_tiles[(b, si)][:, ts(dp, 128)],
                                       ident[:])
                n.scalar.activation(hsT[:, dp, :], tp[:], FT.Copy)
            for si in range(2):
                hs_tiles.pop((b, si))
            qT = lpq.tile([128, KD, SB], bf16, tag="qT", name=f"qT{b}")
            for dp in range(KD):
                qp = ps_u.tile([128, SB], f32, tag="u")
                for k in range(KD):
                    n.tensor.matmul(qp[:], wq_sb[:, k, ts(dp, 128)],
                                    hsT[:, k, :], start=(k == 0),
                                    stop=(k == KD - 1))
                n.scalar.activation(qT[:, dp, :], qp[:], FT.Copy)
            qT_tiles[b] = qT

        fills = list(range(FILLD))

        def maybe_fill():
            if fills:
                b = fills.pop(0)
                emit_fill(b)
                emit_load(b + 2)

        # ================= setup ===========================================
        with tc.tile_pool(name="setup", bufs=1) as setup, \
             tc.tile_pool(name="setup_w", bufs=2) as setup_w:
            encT = setup.tile([128, KC, T], f32r, tag="encT")
            ipT = setup.tile([128, KC, P_IP], f32r, tag="ipT")
            with tc.tile_pool(name="setup_e", bufs=1) as setup_e:
                enc_sb = setup_e.tile([CAT, C], f32, tag="enc")
                n.sync.dma_start(enc_sb[0:T, :], enc_d)
                n.sync.dma_start(enc_sb[IPOFF:CAT, :], ip_d)
                b_row = setup_e.tile([1, D], f32, tag="brow")
                n.sync.dma_start(b_row[:], bout_d[None, :])
                # stage + convert w_q chunks to bf16 (Pool)
                wqf = wq_d.bitcast(f32).rearrange("(ko ki) m -> ki ko m",
                                                  ki=128)
                for k in range(KD):
                    wq_a = setup_w.tile([128, 1024], f32, tag="wk1024")
                    n.sync.dma_start(wq_a[:], wqf[:, k, 0:1024])
                    wq_b = setup_w.tile([128, 256], f32, tag="w256")
                    n.sync.dma_start(wq_b[:], wqf[:, k, 1024:1280])
                    n.gpsimd.tensor_copy(wq_sb[:, k, 0:1024], wq_a[:])
                    n.gpsimd.tensor_copy(wq_sb[:, k, 1024:1280], wq_b[:])
                emit_load(0)
                emit_load(1)
                n.gpsimd.tensor_copy(bias_bf[:], b_row[:])
                for c in range(KC):
                    tpe = ps_u.tile([128, SB], f32, tag="u")
                    n.tensor.transpose(tpe[:, 0:T], enc_sb[0:T, ts(c, 128)],
                                       ident[:T, :T])
                    n.tensor.transpose(tpe[:, 128:128 + P_IP],
                                       enc_sb[IPOFF:CAT, ts(c, 128)],
                                       ident[IPOFF:CAT, IPOFF:IPOFF + P_IP],
                                       tile_position=(IPOFF, 0))
                    n.vector.tensor_copy(encT[:, c, :], tpe[:, 0:T])
                    n.vector.tensor_copy(ipT[:, c, :],
                                         tpe[:, 128:128 + P_IP])

            # ---- txt k/v projections (5-bank psum, one group per bank) --
            # slots: 0 = k[0:512], 1 = k[512:1024], 2 = k[1024:]|v[1024:]
            #        3 = v[0:512], 4 = v[512:1024]
            # slot 2 is ONE matmul group fed by a merged-tail staging tile
            # (psum zero regions are bank-sized: interleaved groups must not
            # share a bank).
            def kv_cloop(sps, kd_, vd_, outp, fill_every):
                for c in range(KC):
                    wk_c = setup_w.tile([128, 1024], f32r, tag="wk1024")
                    n.sync.dma_start(
                        wk_c[:],
                        kd_.rearrange("(co ci) m -> ci co m",
                                      ci=128)[:, c, 0:1024])
                    wv_c = setup_w.tile([128, 1024], f32r, tag="wv1024")
                    n.sync.dma_start(
                        wv_c[:],
                        vd_.rearrange("(co ci) m -> ci co m",
                                      ci=128)[:, c, 0:1024])
                    wt_c = setup_w.tile([128, 512], f32r, tag="wt")
                    n.sync.dma_start(
                        wt_c[:, 0:256],
                        kd_.rearrange("(co ci) m -> ci co m",
                                      ci=128)[:, c, 1024:1280])
                    n.sync.dma_start(
                        wt_c[:, 256:512],
                        vd_.rearrange("(co ci) m -> ci co m",
                                      ci=128)[:, c, 1024:1280])
                    mov = setup_w  # keep staging refs alive
                    for j in range(2):
                        n.tensor.matmul(outp[:, j, :], encT_or(outp)[:, c, :],
                                        wk_c[:, ts(j, 512)],
                                        start=(c == 0), stop=(c == KC - 1))
                        n.tensor.matmul(outp[:, 3 + j, :], encT_or(outp)[:, c, :],
                                        wv_c[:, ts(j, 512)],
                                        start=(c == 0), stop=(c == KC - 1))
                    n.tensor.matmul(outp[:, 2, :], encT_or(outp)[:, c, :],
                                    wt_c[:],
                                    start=(c == 0), stop=(c == KC - 1))
                    if c % 3 == fill_every:
                        maybe_fill()

            def kslot(p, j):
                return p[:, j, 0:512] if j < 2 else p[:, 2, 0:256]

            def vslot(p, j):
                return p[:, 3 + j, 0:512] if j < 2 else p[:, 2, 256:512]

            def drain_kv(p, prows, ksb_flat, voff):
                for j in range(3):
                    w = min(512, D - j * 512)
                    n.scalar.activation(ksb_flat[:, ds(j * 512, w)],
                                        kslot(p, j), FT.Copy)
                with n.allow_low_precision(reason="v row-sums feed small "
                                           "mean correction"):
                    for j in range(3):
                        h0, nh = j * 8, (4 if j == 2 else 8)
                        vj = vslot(p, j).rearrange("p (h c) -> p h c", c=HD)
                        n.vector.tensor_copy(
                            vcat[voff:voff + prows, ds(h0, nh), 0:HD], vj)
                        n.vector.reduce_sum(
                            vcat[voff:voff + prows, ds(h0, nh),
                                 HD + 1:HD + 2], vj,
                            axis=mybir.AxisListType.X)

            encT_or = lambda p: encT if p.shape[0] == T else ipT
            with tc.tile_pool(name="sps1", bufs=1, space="PSUM") as sps1:
                kvp = sps1.tile([T, 5, 512], f32, tag="kvp")
                kv_cloop(sps1, wk_d, wv_d, kvp, 2)
                k_sb = setup.tile([T, KD, 128], f32, tag="ksb")
                drain_kv(kvp, T, k_sb[:].rearrange("p a b -> p (a b)"), 0)
            for dt_ in range(KD):
                tpk = ps_u.tile([128, SB], f32, tag="u")
                n.tensor.transpose(tpk[:, 0:T], k_sb[:, dt_, :], ident[:T, :T])
                n.vector.tensor_copy(ktc_sb[:, dt_, 0:T], tpk[:, 0:T])

            # ---- ip k/v (row-major, same 5-slot psum pattern) -------------
            with tc.tile_pool(name="sps2", bufs=1, space="PSUM") as sps2:
                ikvp = sps2.tile([P_IP, 5, 512], f32, tag="ikvp")
                kv_cloop(sps2, wkip_d, wvip_d, ikvp, 2)
                kip_sb = setup.tile([P_IP, KD, 128], f32, tag="kipsb")
                drain_kv(ikvp, P_IP,
                         kip_sb[:].rearrange("p a b -> p (a b)"), IPOFF)
            for dt_ in range(KD):
                tpk = ps_u.tile([128, SB], f32, tag="u")
                n.tensor.transpose(tpk[:, 0:P_IP], kip_sb[:, dt_, :],
                                   ident[:P_IP, :P_IP])
                n.vector.tensor_copy(ktc_sb[:, dt_, IPOFF:CAT],
                                     tpk[:, 0:P_IP])

            # ---- w_out dma + bf16 convert (Pool) --------------------------
            wof = wout_d.rearrange("(ko ki) m -> ki ko m", ki=128)
            for k in range(KD):
                wo_a = setup_w.tile([128, 1024], f32, tag="wk1024")
                n.sync.dma_start(wo_a[:], wof[:, k, 0:1024])
                wo_b = setup_w.tile([128, 256], f32, tag="w256")
                n.sync.dma_start(wo_b[:], wof[:, k, 1024:1280])
                n.gpsimd.tensor_copy(wout_bf[:, k, 0:1024], wo_a[:])
                n.gpsimd.tensor_copy(wout_bf[:, k, 1024:1280], wo_b[:])
                if k % 4 == 3:
                    maybe_fill()
            while fills:
                maybe_fill()

        # ================= main loop =======================================
        ps_pv = ctx.enter_context(tc.tile_pool(name="ps_pv", bufs=2,
                                               space="PSUM"))
        ps_out = ctx.enter_context(tc.tile_pool(name="ps_out", bufs=2,
                                                space="PSUM"))
        state = {}

        def emit_attn(b):
            """scores+exp (packed txt|ip), PV, normalize, stats for block b;
            out-projection chunks of block b-1 are interleaved into the
            score loop."""
            qT = qT_tiles.pop(b)
            pT = {}
            for h in range(H):
                dt_, half = h // 2, h % 2
                sc = ps_u.tile([CAT, SB], f32, tag="u")
                n.tensor.matmul(sc[:], ktc_sb[ds(64 * half, 64), dt_, :],
                                qT[ds(64 * half, 64), dt_, :],
                                start=True, stop=True)
                pT[h] = lpp.tile([CAT, SB], bf16, tag="pT", name=f"pT{h}")
                n.scalar.activation(pT[h][:], sc[:], FT.Exp, scale=SCALE)
                if h % 3 == 2 and state.get("out_chunks"):
                    state["out_chunks"].pop(0)()
            while state.get("out_chunks"):
                state["out_chunks"].pop(0)()

            lat = lp1.tile([128, 2, D], bf16, tag="lat", name=f"lat{b}")
            ipo = lp1.tile([128, 2, D], bf16, tag="ipo", name=f"ipo{b}")
            msum_l = lps.tile([128, 2, H], f32, tag="msl")
            msum_i = lps.tile([128, 2, H], f32, tag="msi")
            recip_l = lps.tile([128, 2, H], f32, tag="rcl")
            recip_i = lps.tile([128, 2, H], f32, tag="rci")
            st = lps.tile([128, 2, 16], f32, tag="st", name=f"st{b}")

            BANKS = [list(range(6 * g, min(H, 6 * g + 6))) for g in range(4)]
            for g, bank in enumerate(BANKS):
                nb = len(bank)
                for si in range(2):
                    for br in range(2):  # 0 = txt, 1 = ip
                        pv = ps_pv.tile([128, 512], f32, tag="pv")
                        r0 = 0 if br == 0 else IPOFF
                        r1 = T if br == 0 else CAT
                        for bi, h in enumerate(bank):
                            n.tensor.matmul(
                                pv[:, ds(bi * VW, VW)],
                                pT[h][r0:r1, ts(si, 128)],
                                vcat[r0:r1, h, :], start=True, stop=True,
                                tile_position=(r0, 0))
                        pv3 = pv[:, :nb * VW].rearrange("p (h c) -> p h c",
                                                        c=VW)
                        recip = recip_l if br == 0 else recip_i
                        msum = msum_l if br == 0 else msum_i
                        dest = lat if br == 0 else ipo
                        n.vector.reciprocal(recip[:, si, ds(6 * g, nb)],
                                            pv3[:, :, HD])
                        n.vector.tensor_mul(msum[:, si, ds(6 * g, nb)],
                                            pv3[:, :, HD + 1],
                                            recip[:, si, ds(6 * g, nb)])
                        with n.allow_low_precision(reason="attn out bf16"):
                            n.vector.tensor_tensor(
                                dest[:, si, ds(6 * g * HD, nb * HD)].rearrange(
                                    "p (h c) -> p h c", c=HD),
                                pv3[:, :, 0:HD],
                                recip[:, si, ds(6 * g, nb), None].to_broadcast(
                                    [128, nb, HD]),
                                op=ALU.mult)

            # ---- norm_ipa stats ----
            scr = lscr.tile([128, D], bf16, tag="scr")
            for si in range(2):
                n.vector.reduce_sum(st[:, si, 0:1], msum_l[:, si, :],
                                    axis=mybir.AxisListType.X)
                n.vector.reduce_sum(st[:, si, 1:2], msum_i[:, si, :],
                                    axis=mybir.AxisListType.X)
                with n.allow_low_precision(reason="scratch for accum"):
                    n.scalar.activation(scr[:], lat[:, si, :], FT.Square,
                                        accum_out=st[:, si, 2:3])
                    n.scalar.activation(scr[:], ipo[:, si, :], FT.Square,
                                        accum_out=st[:, si, 3:4])
                n.vector.tensor_scalar_mul(st[:, si, 4:5], st[:, si, 0:1],
                                           1.0 / D)
                n.vector.tensor_scalar_mul(st[:, si, 5:6], st[:, si, 1:2],
                                           1.0 / D)
                n.vector.tensor_mul(st[:, si, 6:7], st[:, si, 4:5],
                                    st[:, si, 4:5])
                n.vector.tensor_mul(st[:, si, 7:8], st[:, si, 5:6],
                                    st[:, si, 5:6])
                n.vector.tensor_scalar(out=st[:, si, 8:9], in0=st[:, si, 2:3],
                                       scalar1=1.0 / D, scalar2=st[:, si, 6:7],
                                       op0=ALU.mult, op1=ALU.subtract)
                n.vector.tensor_scalar(out=st[:, si, 9:10], in0=st[:, si, 3:4],
                                       scalar1=1.0 / D, scalar2=st[:, si, 7:8],
                                       op0=ALU.mult, op1=ALU.subtract)
            # std = var * rsqrt(var): fast-inverse-sqrt init + 3 Newton iters
            i32 = mybir.dt.int32
            vv = st[:, :, 8:10]
            yy = st[:, :, 10:12]
            t0 = st[:, :, 12:14]
            n.vector.tensor_scalar(out=yy.bitcast(i32), in0=vv.bitcast(i32),
                                   scalar1=1, scalar2=None,
                                   op0=ALU.logical_shift_right)
            n.vector.tensor_scalar(out=yy.bitcast(i32), in0=yy.bitcast(i32),
                                   scalar1=-1, scalar2=0x5f3759df,
                                   op0=ALU.mult, op1=ALU.add)
            for _ in range(3):
                n.vector.tensor_mul(t0[:], yy[:], yy[:])
                n.vector.tensor_mul(t0[:], t0[:], vv[:])
                n.vector.tensor_scalar(out=t0[:], in0=t0[:], scalar1=-0.5,
                                       scalar2=1.5, op0=ALU.mult, op1=ALU.add)
                n.vector.tensor_mul(yy[:], yy[:], t0[:])
            n.vector.tensor_mul(yy[:], vv[:], yy[:])  # std = var * rsqrt(var)
            for si in range(2):
                n.vector.tensor_scalar_add(st[:, si, 12:13], st[:, si, 11:12],
                                           EPS)
                n.vector.reciprocal(st[:, si, 13:14], st[:, si, 12:13])
                n.vector.tensor_mul(st[:, si, 14:15], st[:, si, 10:11],
                                    st[:, si, 13:14])
                # gneg = alpha*mean_ip - mean_lat
                n.vector.scalar_tensor_tensor(
                    out=st[:, si, 15:16], in0=st[:, si, 5:6],
                    scalar=st[:, si, 14:15], in1=st[:, si, 4:5],
                    op0=ALU.mult, op1=ALU.subtract)
                # hs_sum = lat + alpha*ip - gneg   (bf16, DVE 2x mode)
                with n.allow_low_precision(reason="hs_sum bf16"):
                    n.vector.scalar_tensor_tensor(
                        out=lat[:, si, :], in0=ipo[:, si, :],
                        scalar=st[:, si, 14:15], in1=lat[:, si, :],
                        op0=ALU.mult, op1=ALU.add)
                    n.vector.tensor_scalar_sub(lat[:, si, :], lat[:, si, :],
                                               st[:, si, 15:16])
            state[b] = lat

        def emit_tr2(b):
            """transpose hs_sum(b) -> hsT2 (bf16) for the out projection."""
            lat = state.pop(b)
            hsT2 = lp1.tile([128, KD, SB], bf16, tag="hsT2", name=f"hsT2{b}")
            for dp in range(0, KD, 2):
                tpb = ps_u.tile([128, 512], bf16, tag="u")
                for dd in range(2):
                    for si in range(2):
                        n.tensor.transpose(
                            tpb[:, ds(dd * 256 + si * 128, 128)],
                            lat[:, si, ts(dp + dd, 128)], ident_bf[:])
                n.vector.tensor_copy(hsT2[:, dp:dp + 2, :], tpb[:])
            return hsT2

        def make_out_chunks(b, hsT2):
            """closures for the 6 out-projection (si, j) groups of block b."""
            chunks = []
            s0 = b * SB

            def mk(si, j):
                def go():
                    w = min(512, D - j * 512)
                    op = ps_out.tile([128, 512], f32, tag="out")
                    for k in range(KD):
                        n.tensor.matmul(op[:, :w], hsT2[:, k, ts(si, 128)],
                                        wout_bf[:, k, ds(j * 512, w)],
                                        start=(k == 0), stop=False)
                    n.tensor.matmul(op[:, :w], ones_bf[:],
                                    bias_bf[:, ds(j * 512, w)],
                                    start=False, stop=True)
                    ost = lpo.tile([128, 512], f32, tag="ost")
                    n.vector.tensor_copy(ost[:, :w], op[:, :w])
                    n.sync.dma_start(
                        out_d[ds(s0 + si * 128, 128), ds(j * 512, w)],
                        ost[:, :w])
                return go

            for si in range(2):
                for j in range(3):
                    chunks.append(mk(si, j))
            return chunks

        for i in range(NBLK + 1):
            if i >= 1:
                hsT2 = emit_tr2(i - 1)
                state["out_chunks"] = make_out_chunks(i - 1, hsT2)
            if i < NBLK:
                emit_attn(i)
                emit_fill(i + FILLD)
                emit_load(i + FILLD + 2)
            else:
                while state.get("out_chunks"):
                    state["out_chunks"].pop(0)()
    nc.compile()
    return nc


def _get_nc():
    if "nc" not in _CACHE:
        _CACHE["nc"] = _build()
    return _CACHE["nc"]


def kernel(**inputs) -> np.ndarray:
    nc = _get_nc()
    f = lambda x: np.ascontiguousarray(np.asarray(x), dtype=np.float32)
    shared = {k: f(inputs[k]) for k in
              ("w_q", "w_k", "w_v", "w_k_ip", "w_v_ip", "w_out", "b_out")}
    hs = f(inputs["hidden_states"])
    enc = f(inputs["encoder_hidden_states"])
    ipx = f(inputs["ip_hidden_states"])
    in_maps = [
        dict(shared, hidden_states=hs[i], encoder_hidden_states=enc[i],
             ip_hidden_states=ipx[i])
        for i in range(8)
    ]
    res = bass_utils.run_bass_kernel_spmd(nc, in_maps, core_ids=list(range(8)))
    return np.stack([res.results[i]["out"] for i in range(8)], axis=0)


if __name__ == "__main__":
    rng = np.random.default_rng(0)
    ins = {
        "hidden_states": rng.standard_normal((B, S, D), dtype=np.float32),
        "encoder_hidden_states": rng.standard_normal((B, T, C), dtype=np.float32),
        "ip_hidden_states": rng.standard_normal((B, P_IP, C), dtype=np.float32),
        "w_q": (rng.standard_normal((D, D), dtype=np.float32) * 0.02),
        "w_k": (rng.standard_normal((C, D), dtype=np.float32) * 0.02),
        "w_v": (rng.standard_normal((C, D), dtype=np.float32) * 0.02),
        "w_k_ip": (rng.standard_normal((C, D), dtype=np.float32) * 0.02),
        "w_v_ip": (rng.standard_normal((C, D), dtype=np.float32) * 0.02),
        "w_out": (rng.standard_normal((D, D), dtype=np.float32) * 0.02),
        "b_out": np.zeros((D,), dtype=np.float32),
    }
    out = kernel(**ins)
    print("out", out.shape, out.dtype, float(np.abs(out).max()))
